# revision 18
# baseline (speedup 1.0000x reference)
"""BoxFilter 9x9 mean, TRN2 x8 — v6: f16 host-pad input, batched HWDGE loads,
SWDGE stores, folded normalization, 2-image pipelining.

Host: x is cast to f16 and zero-padded horizontally to 1036 cols (4 left,
8 right), so the device does no memsets and no casting (input DMAs ride the
HWDGE path on the idle sync queue instead of gpsimd SWDGE desc-gen).

Device, per image: one [128, 9, 1036] SBUF tile holds nine 128-row input
blocks (rows re-read across block halos). Per block: two DVE shift-adds
build e2 (4-tap horizontal sums), then per 512-col chunk three PE matmuls
against a band weight (vertical 9-sum x horizontal completion) accumulate
into a 2-bank PSUM tile; the band weights carry the 1/(9*count_v)
normalization so one ScalarE copy per block evacuates PSUM->SBUF f16.
Outputs leave via two gpsimd (SWDGE) DMAs per image, keeping the scalar
queue free of DMA stalls. Horizontal edge-count fix-up happens on host.
"""

import threading

import numpy as np

NCORES = 8
B, C, H, W = 16, 3, 1024, 1024
IMGS = B * C
IMGS_PER_CORE = IMGS // NCORES
R = 4
OB = 120  # output rows per full block
NFULL = H // OB  # 8 full blocks
LASTO = H - NFULL * OB  # 64
WP = W + 8  # 4-col zero pad each side

# per-image block table: (out_start, out_rows, in_start, in_rows, w_idx)
BLOCKS = []
BLOCKS.append((0, OB, 0, 124, 0))
for I in range(1, NFULL):
    BLOCKS.append((OB * I, OB, OB * I - R, 128, 1))
BLOCKS.append((H - LASTO, LASTO, H - 96, 96, 2))


def _window_counts():
    r = np.arange(H)
    return (np.minimum(r + R, H - 1) - np.maximum(r - R, 0) + 1).astype(np.float32)


def _consts():
    ch = _window_counts()
    k = np.arange(128)[:, None]
    m = np.arange(128)[None, :]
    # W0: tile rows = image rows 0..123; out m needs rows max(0,m-4)..m+4
    w0 = ((np.maximum(m - R, 0) <= k) & (k <= m + R) & (m < OB)).astype(np.float32)
    # W_int: tile rows = image rows s-4..s+123; out m needs tile k = m..m+8
    wi = ((m <= k) & (k <= m + 2 * R) & (m < OB)).astype(np.float32)
    # W8: tile rows = image rows 928..1023 (96); out m (0..63, global 960+m)
    # needs k = 28+m .. min(36+m, 95)
    w8 = ((m + 32 - R <= k) & (k <= np.minimum(m + 32 + R, 95)) & (m < LASTO)).astype(
        np.float32
    )
    # Fold 1/(9*count_v) normalization into the weights (host fixes the
    # horizontal edge columns afterwards).
    s0 = np.where(np.arange(128) < OB, 1.0 / (9.0 * ch[0:128]), 0.0)
    w0 *= s0[None, :]
    wi *= 1.0 / 81.0
    s8 = np.zeros(128, np.float32)
    s8[0:LASTO] = 1.0 / (9.0 * ch[H - LASTO : H])
    w8 *= s8[None, :]
    return np.stack([w0, wi, w8]).astype(np.float16)


def _build(reps: int = 1):
    import concourse.bacc as bacc
    import concourse.bass as bass
    import concourse.mybir as mybir
    import concourse.tile as tile

    f32 = mybir.dt.float32
    f16 = mybir.dt.float16

    nc = bacc.Bacc("TRN2", target_bir_lowering=False, debug=False, num_devices=NCORES)
    x_d = nc.declare_dram_parameter("x", [IMGS_PER_CORE, H, WP], f16, isOutput=False)
    wts_d = nc.declare_dram_parameter("wts", [3, 128, 128], f16, isOutput=False)
    o_d = nc.declare_dram_parameter("out", [IMGS_PER_CORE, H, W], f16, isOutput=True)

    with tile.TileContext(nc) as tc:
        with (
            tc.tile_pool(name="consts", bufs=1) as cpool,
            tc.tile_pool(name="xb", bufs=5) as x_pool,
            tc.tile_pool(name="e1", bufs=8) as e1_pool,
            tc.tile_pool(name="e2", bufs=8) as e2_pool,
            tc.tile_pool(name="osb", bufs=3) as o_pool,
            tc.tile_pool(name="e8", bufs=3) as e8_pool,
            tc.tile_pool(name="ps", bufs=4, space="PSUM") as ps_pool,
        ):
            w_sb = cpool.tile([128, 3 * 128], f16)
            for i in range(3):
                nc.scalar.dma_start(out=w_sb[:, 128 * i : 128 * (i + 1)], in_=wts_d[i])

            # Warm the PE p-state during the input-DMA fill: ~3us of dummy
            # matmuls so the first real matmuls run at full clock.
            dummy_w = cpool.tile([128, 128], f16)
            dummy_r = cpool.tile([128, 512], f16)
            nc.vector.memset(dummy_w[:, :], 0.0)
            nc.vector.memset(dummy_r[:, :], 0.0)
            ps_warm = ps_pool.tile([128, 1024], f32, tag="ps", name="ps_warm")
            for _w in range(8):
                nc.tensor.matmul(
                    ps_warm[:, 0:512], dummy_w[:, :], dummy_r[:, :],
                    start=True, stop=True,
                )

            def emit_mid(xb, g, b0, b1):
                # blocks b0..b1-1: rows 120b-4 .. 120b+123, strided-overlapping
                ref = x_d[g, b0 * OB - R : b0 * OB - R + 128, :]
                mid = bass.AP(
                    ref.tensor, ref.offset, [[WP, 128], [OB * WP, b1 - b0], [1, WP]]
                )
                nc.sync.dma_start(out=xb[:, b0:b1, :], in_=mid)

            def emit_input(g, fine=False):
                xb = x_pool.tile([128, 9, WP], f16, tag="xb", name=f"xb{g}")
                # block 0: image rows 0..123
                nc.sync.dma_start(out=xb[0:124, 0, :], in_=x_d[g, 0:124, :])
                # split so compute can start before the whole image lands; the
                # first image is split finest since nothing overlaps it
                for b0, b1 in ((1, 2), (2, 3), (3, 4), (4, 6), (6, 8)) if fine else (
                    (1, 4),
                    (4, 8),
                ):
                    emit_mid(xb, g, b0, b1)
                # block 8: rows 928..1023
                nc.sync.dma_start(out=xb[0:96, 8, :], in_=x_d[g, H - 96 : H, :])
                return xb

            def block(xb, out_sb, blk, split_copy=False, pool_e8=False):
                os_, orows, is_, irows, wi_ = blk
                b = BLOCKS.index(blk)
                e1 = e1_pool.tile([128, WP - 2], f16, tag="e1")
                nc.vector.tensor_add(
                    out=e1[0:irows, :],
                    in0=xb[0:irows, b, 0 : WP - 2],
                    in1=xb[0:irows, b, 2:WP],
                )
                e2 = e2_pool.tile([128, WP - 6], f16, tag="e2")
                nc.vector.tensor_add(
                    out=e2[0:irows, :],
                    in0=e1[0:irows, 0 : WP - 6],
                    in1=e1[0:irows, 4 : WP - 2],
                )
                if pool_e8:
                    # 8-tap horizontal sum on the otherwise-idle gpsimd engine
                    # trades one matmul per chunk for Pool time
                    e8 = e8_pool.tile([128, W], f16, tag="e8")
                    nc.gpsimd.tensor_add(
                        out=e8[0:irows, :],
                        in0=e2[0:irows, 0:W],
                        in1=e2[0:irows, 1 : W + 1],
                    )
                wv = w_sb[0:irows, 128 * wi_ : 128 * wi_ + orows]
                ps = ps_pool.tile([128, 1024], f32, tag="ps", name=f"ps{b}")
                for h in range(2):
                    j0 = 512 * h
                    if pool_e8:
                        nc.tensor.matmul(
                            ps[0:orows, j0 : j0 + 512],
                            wv,
                            e8[0:irows, j0 : j0 + 512],
                            start=True,
                            stop=False,
                        )
                    else:
                        nc.tensor.matmul(
                            ps[0:orows, j0 : j0 + 512],
                            wv,
                            e2[0:irows, j0 : j0 + 512],
                            start=True,
                            stop=False,
                        )
                        nc.tensor.matmul(
                            ps[0:orows, j0 : j0 + 512],
                            wv,
                            e2[0:irows, j0 + 1 : j0 + 513],
                            start=False,
                            stop=False,
                        )
                    nc.tensor.matmul(
                        ps[0:orows, j0 : j0 + 512],
                        wv,
                        xb[0:irows, b, j0 + 8 : j0 + 520],
                        start=False,
                        stop=True,
                    )
                if split_copy:
                    nc.scalar.copy(out_sb[0:orows, b, 0:512], ps[0:orows, 0:512])
                    nc.scalar.copy(out_sb[0:orows, b, 512:1024], ps[0:orows, 512:1024])
                else:
                    nc.scalar.copy(out_sb[0:orows, b, :], ps[0:orows, :])

            for _ in range(reps):
                xbs = {0: emit_input(0, fine=True)}
                if IMGS_PER_CORE > 1:
                    xbs[1] = emit_input(1)
                for g in range(IMGS_PER_CORE):
                    last = g == IMGS_PER_CORE - 1
                    xb = xbs.pop(g)
                    out_sb = o_pool.tile([128, 9, W], f16, tag="osb", name=f"osb{g}")
                    if not last:
                        for b, blk in enumerate(BLOCKS):
                            block(xb, out_sb, blk)
                            if b % 2 == 1:
                                # stream outputs: rows 120b+p per pair of blocks
                                ref = o_d[g, OB * (b - 1) : OB * (b + 1), :]
                                pair = bass.AP(
                                    ref.tensor,
                                    ref.offset,
                                    [[W, OB], [OB * W, 2], [1, W]],
                                )
                                nc.gpsimd.dma_start(
                                    out=pair, in_=out_sb[0:OB, b - 1 : b + 1, :]
                                )
                        if g + 2 < IMGS_PER_CORE:
                            xbs[g + 2] = emit_input(g + 2)
                        nc.gpsimd.dma_start(
                            out=o_d[g, H - LASTO : H, :], in_=out_sb[0:LASTO, 8, :]
                        )
                    else:
                        # drain phase is compute-paced: small block 8 first,
                        # then stream per block; the tail block 7 streams per
                        # 512-col chunk
                        block(xb, out_sb, BLOCKS[8])
                        nc.gpsimd.dma_start(
                            out=o_d[g, H - LASTO : H, :], in_=out_sb[0:LASTO, 8, :]
                        )
                        for b in range(8):
                            block(xb, out_sb, BLOCKS[b], split_copy=b == 7)
                            if b < 7:
                                nc.gpsimd.dma_start(
                                    out=o_d[g, OB * b : OB * (b + 1), :],
                                    in_=out_sb[0:OB, b, :],
                                )
                            else:
                                for h in range(2):
                                    j0 = 512 * h
                                    nc.gpsimd.dma_start(
                                        out=o_d[g, OB * 7 : OB * 8, j0 : j0 + 512],
                                        in_=out_sb[0:OB, 7, j0 : j0 + 512],
                                    )

    nc.compile()
    return nc


_LOCK = threading.Lock()
_CACHED = {}


def _get_nc(reps: int = 1):
    with _LOCK:
        key = ("nc", reps)
        if key not in _CACHED:
            _CACHED[key] = _build(reps)
        return _CACHED[key]


def _postprocess(out48_f16: np.ndarray) -> np.ndarray:
    out = out48_f16.astype(np.float32).reshape(B, C, H, W)
    ch = _window_counts()
    out[..., 0:R] *= (9.0 / ch[0:R])[None, None, None, :]
    out[..., W - R : W] *= (9.0 / ch[H - R : H])[None, None, None, :]
    return out


def run(x: np.ndarray, trace: bool = False, reps: int = 1):
    from concourse.bass_utils import run_bass_kernel_spmd

    assert x.shape == (B, C, H, W), x.shape
    x48 = np.asarray(x, dtype=np.float32).reshape(IMGS, H, W)
    xpad = np.zeros((IMGS, H, WP), np.float16)
    xpad[:, :, R : R + W] = x48
    wts = _consts()
    in_maps = [
        {
            "x": np.ascontiguousarray(
                xpad[IMGS_PER_CORE * c : IMGS_PER_CORE * (c + 1)]
            ),
            "wts": wts,
        }
        for c in range(NCORES)
    ]
    nc = _get_nc(reps)
    res = run_bass_kernel_spmd(
        nc, in_maps, core_ids=list(range(NCORES)), trace=trace
    )
    out48 = np.concatenate([r["out"] for r in res.results], axis=0)
    return _postprocess(out48), res


def kernel(x: np.ndarray) -> np.ndarray:
    out, _ = run(x, trace=False)
    return out
